# revision 4
# baseline (speedup 1.0000x reference)
"""Trainium2 Bass kernel for the cos/sin broadcast-multiply problem.

reference:
    a_vals[j] = 2*pi*freq_init[0] * (-j) * dt      (dt == (t[-1]-t[0])/511, t = arange(512)/30)
    real = cos(a_vals)[:, None, None] * x          x: [512, 3, 32768] f32
    imag = sin(a_vals)[:, None, None] * x
    returns (real, imag)

Strategy: the length-512 cos/sin vectors are computed on host (trivially small);
the 201 MB broadcast multiply runs on 8 NeuronCores, data-parallel along the
S (=32768) axis.  Each core gets x_shard [512, 3*4096] = [512, 12288] and
produces real/imag shards of the same shape.  On-device it is a pure
streaming kernel: DMA in [128, FT] tiles, one per-partition-scalar multiply
on the scalar engine (imag) and one on the vector engine (real, in place),
DMA both results out.
"""

import numpy as np

N_CORES = 8
N = 512          # window length (partition-tiled 4 x 128)
C = 3
S = 32768
S_SH = S // N_CORES          # 4096 per core
CW = C * S_SH                # 12288 free-dim columns per core
FT = 6144                    # free-dim tile width (3 MB DMA transfers)
P = 128

_nc_cache = None


def _build_nc():
    """Build the Bass module (one NeuronCore's program, SPMD across 8)."""
    import concourse.bacc as bacc
    import concourse.mybir as mybir
    from concourse.tile import TileContext

    F32 = mybir.dt.float32

    nc = bacc.Bacc()
    x = nc.dram_tensor("x", [N, CW], F32, kind="ExternalInput")
    # trig[p, pi]   = cos[pi*128 + p]  for pi in 0..3
    # trig[p, 4+pi] = sin[pi*128 + p]
    trig = nc.dram_tensor("trig", [P, 8], F32, kind="ExternalInput")
    out_r = nc.dram_tensor("out_r", [N, CW], F32, kind="ExternalOutput")
    out_i = nc.dram_tensor("out_i", [N, CW], F32, kind="ExternalOutput")

    with TileContext(nc) as tc:
        with (
            tc.tile_pool(name="const", bufs=1) as cpool,
            tc.tile_pool(name="xp", bufs=4) as xpool,
            tc.tile_pool(name="ip", bufs=3) as ipool,
        ):
            # trig via SWDGE (gpsimd) so the SP HWDGE ring starts with x loads
            trig_t = cpool.tile([P, 8], F32)
            nc.gpsimd.dma_start(out=trig_t[:], in_=trig[:])
            for pi in range(N // P):
                rows = slice(pi * P, (pi + 1) * P)
                for fj in range(CW // FT):
                    cols = slice(fj * FT, (fj + 1) * FT)
                    xt = xpool.tile([P, FT], F32, tag="x")
                    it = ipool.tile([P, FT], F32, tag="imag")
                    # loads on the SP HWDGE ring only
                    nc.sync.dma_start(out=xt[:], in_=x[rows, cols])
                    # both multiplies on the vector engine (2x mode, ~3.4us each)
                    nc.vector.tensor_scalar_mul(it[:], xt[:], trig_t[:, 4 + pi : 5 + pi])
                    nc.vector.tensor_scalar_mul(xt[:], xt[:], trig_t[:, pi : pi + 1])
                    # stores on the ACT HWDGE ring only
                    nc.scalar.dma_start(out=out_i[rows, cols], in_=it[:])
                    nc.scalar.dma_start(out=out_r[rows, cols], in_=xt[:])
    nc.finalize()
    return nc


def _trig_table(freq_init: np.ndarray) -> np.ndarray:
    """Replicate the reference's f32 arithmetic for a_vals, then cos/sin."""
    f = np.float32(np.asarray(freq_init).reshape(-1)[0])
    t = np.arange(N, dtype=np.float32) / np.float32(30.0)
    dt = (t[-1] - t[0]) / np.float32(N - 1)
    k = np.arange(N, dtype=np.float32)
    a = np.float32(2.0 * np.pi) * f
    a = a * (-k)
    a = a * dt  # f32 [512], bit-matching the jnp chain
    a64 = a.astype(np.float64)
    cos = np.cos(a64).astype(np.float32)
    sin = np.sin(a64).astype(np.float32)
    trig = np.empty((P, 8), dtype=np.float32)
    for pi in range(N // P):
        trig[:, pi] = cos[pi * P : (pi + 1) * P]
        trig[:, 4 + pi] = sin[pi * P : (pi + 1) * P]
    return trig


def run(x: np.ndarray, freq_init: np.ndarray, trace: bool = False):
    """Run on 8 NeuronCores. Returns ((real, imag), exec_time_ns|None)."""
    global _nc_cache
    from concourse.bass_utils import run_bass_kernel_spmd

    x = np.asarray(x)
    assert x.shape == (N, C, S) and x.dtype == np.float32, (x.shape, x.dtype)

    if _nc_cache is None:
        _nc_cache = _build_nc()
    nc = _nc_cache

    trig = _trig_table(freq_init)
    in_maps = []
    for i in range(N_CORES):
        shard = np.ascontiguousarray(x[:, :, i * S_SH : (i + 1) * S_SH]).reshape(N, CW)
        in_maps.append({"x": shard, "trig": trig})

    res = run_bass_kernel_spmd(nc, in_maps, list(range(N_CORES)), trace=trace)

    real = np.concatenate(
        [r["out_r"].reshape(N, C, S_SH) for r in res.results], axis=2
    )
    imag = np.concatenate(
        [r["out_i"].reshape(N, C, S_SH) for r in res.results], axis=2
    )
    return (real, imag), res.exec_time_ns


def kernel(x: np.ndarray, freq_init: np.ndarray):
    (real, imag), _ = run(x, freq_init, trace=False)
    return real, imag
